# revision 41
# baseline (speedup 1.0000x reference)
"""Trainium2 Bass kernel for nn_AIGMAEEncoder (DeepGCN/GENConv encoder).

G=8 graphs sharded 1 graph/NeuronCore (data parallel, no collectives).

Per-core algorithm per layer (engine-balanced rewrite):
  node phase (per 4-chunk superblock, interleaved into previous layer's MLP):
    bn_stats/aggr -> mu,var; rln=1/sqrt(var+eps) (scalar Sqrt + vec recip)
    h = (x-mu)*rln            (vector tensor_scalar, kept in SBUF all layer)
    m = Relu(rln*x - mu*rln)  (scalar activation, AP scale/bias)
    em = exp(t*m)             (scalar) ; mm = m*em (vector)
    [em|mm] -> rbuf[l%2] DRAM (1KB rows, HWDGE)
  edge+MLP phase (per 4-chunk group):
    dma_gather of dst-sorted edge slots (tight packing: per-group pad to 128,
    boundary subchunks matmul into both chunks' PSUM accumulations)
    S = eps_bump + sum_k amat_k^T @ et_k   (PE; eps bump makes S1>0 for
        empty dst nodes so ag=S2/S1 -> 0 without clamps)
    r1 = recip_approx(S1); ag = S2*r1; z = ag + h
    zT via PE transposes; yc = w1c^T zT (mean folded into w1c)
    var = rowdot(zT^T gram, z)/512 (tensor_tensor_reduce); rinv=1/sqrt(var+eps)
    o = rinv * (relu(yc) @ w2g); x += o; relu between layers
"""
import sys
import numpy as np
import ml_dtypes

for _p in ("/opt/trn_rl_repo",):
    if _p not in sys.path:
        sys.path.insert(0, _p)

BF16 = ml_dtypes.bfloat16
G, N, E, H, L = 8, 8192, 16384, 256, 4
H2 = 2 * H
P = 128
NCHUNK = N // P           # 64
GRP = 4                   # node-chunks per group (=512 nodes)
NGRP = NCHUNK // GRP      # 16
GSZ = GRP * P             # 512 nodes per group
EPS = 1e-7
LN_EPS = 1e-5
EPS_V = 2.0 ** -20        # per-row value of the S1 epsilon-bump matmul


# ----------------------------------------------------------------- host prep
def _prep_edges(input_edges):
    """Tightly pack dst-sorted edges into 128-slot subchunks per 4-chunk
    group. Returns per-group subchunk counts, the per-chunk matmul lists
    (subchunks may span chunk boundaries -> contribute to 2+ chunks), the
    gather index tensor, and the one-hot scatter matrices."""
    N2 = N // 2
    NSPLIT = 4  # groups whose gather is split by src half (early groups)
    src = np.asarray(input_edges[:, 0], np.int64)   # [G, E]
    dst = np.asarray(input_edges[:, 1], np.int64)
    # sort by (group, [src-half for split groups], dst): split groups get a
    # src<N/2 prefix and a src>=N/2 suffix, each dst-sorted
    grp_of = dst // GSZ
    half_of = np.where(grp_of < NSPLIT, src // N2, 0)
    skey = grp_of * (2 * N) + half_of * N + dst
    order = np.argsort(skey, axis=1, kind="stable")
    src_s = np.take_along_axis(src, order, axis=1)
    dst_s = np.take_along_axis(dst, order, axis=1)

    # per (graph, group, half) edge counts
    e_grh = np.zeros((G, NGRP, 2), np.int64)
    for g in range(G):
        for r in range(NGRP):
            in_r = grp_of[g] == r
            if r < NSPLIT:
                e_grh[g, r, 0] = int(np.sum(in_r & (src[g] < N2)))
                e_grh[g, r, 1] = int(np.sum(in_r & (src[g] >= N2)))
            else:
                e_grh[g, r, 0] = int(np.sum(in_r))
    gnh = np.maximum(1, -(-e_grh.max(axis=0) // P))  # [NGRP, 2]
    gnh[NSPLIT:, 1] = 0
    gn = gnh.sum(axis=1)                             # [NGRP]
    sub_off = np.concatenate([[0], np.cumsum(gn)])   # group subchunk offsets
    total_sub = int(sub_off[-1])
    total_slots = total_sub * P

    idx_pad = np.zeros((G, total_slots), np.int16)
    # (sub, chunk) presence union over graphs
    present = [set() for _ in range(total_sub)]
    per_graph = []
    for g in range(G):
        rows = []
        pos = 0
        for r in range(NGRP):
            nA = int(e_grh[g, r, 0])
            nB = int(e_grh[g, r, 1])
            for half, nreal in ((0, nA), (1, nB)):
                b0 = pos
                pos += nreal
                s_base = int(sub_off[r]) + (int(gnh[r, 0]) if half else 0)
                s0 = s_base * P
                srcs = src_s[g, b0:b0 + nreal] - (N2 if half else 0)
                idx_pad[g, s0:s0 + nreal] = srcs
                j = np.arange(nreal)
                subs = s_base + j // P
                parts = j % P
                chunks = dst_s[g, b0:b0 + nreal] // P
                dloc = dst_s[g, b0:b0 + nreal] % P
                if nreal:
                    for sidx, cidx in zip(*np.unique(
                            np.stack([subs, chunks]), axis=1)):
                        present[int(sidx)].add(int(cidx))
                rows.append((subs, parts, chunks, dloc))
        per_graph.append(rows)

    # assign matrix indices per (sub, chunk) pair; mats are contiguous per
    # group (subchunks only hold edges of their own group). chunk_mats holds
    # (group-local mat idx, group-local sub idx).
    mat_idx = {}
    chunk_mats = [[] for _ in range(NCHUNK)]
    for s in range(total_sub):
        for c in sorted(present[s]):
            mat_idx[(s, c)] = len(mat_idx)
    nmats = len(mat_idx)
    g_mat_off = [nmats] * (NGRP + 1)
    for (s, c), mi in mat_idx.items():
        r = int(np.searchsorted(sub_off, s, side="right")) - 1
        g_mat_off[r] = min(g_mat_off[r], mi)
        chunk_mats[c].append((mi, s - int(sub_off[r])))
    g_mat_off[NGRP] = nmats
    for r in range(NGRP - 1, -1, -1):
        g_mat_off[r] = min(g_mat_off[r], g_mat_off[r + 1])
    for c in range(NCHUNK):
        chunk_mats[c].sort()
        chunk_mats[c] = [(mi - g_mat_off[c // GRP], si)
                         for mi, si in chunk_mats[c]]

    amat = np.zeros((G, P, nmats, P), BF16)
    for g in range(G):
        for subs, parts, chunks, dloc in per_graph[g]:
            for s, p_, c, d in zip(subs, parts, chunks, dloc):
                amat[g, p_, mat_idx[(int(s), int(c))], d] = 1.0

    idxw = np.zeros((G, P, total_slots // 16), np.int16)
    for g in range(G):
        w = idx_pad[g].reshape(-1, 16).T
        idxw[g] = np.tile(w, (8, 1))

    return dict(gn=[int(v) for v in gn],
                gnh=[(int(a), int(b)) for a, b in gnh],
                sub_off=[int(v) for v in sub_off],
                total_sub=total_sub, nmats=nmats,
                g_mat_off=[int(v) for v in g_mat_off],
                chunk_mats=chunk_mats, idxw=idxw, amat=amat)


def _prep_weights(inputs):
    """Fold trivial affines; raise AssertionError if assumptions break."""
    ln_g, ln_b = np.asarray(inputs["ln_g"]), np.asarray(inputs["ln_b"])
    mg, mb = np.asarray(inputs["mlp_ln_g"]), np.asarray(inputs["mlp_ln_b"])
    b1, b2 = np.asarray(inputs["b1"]), np.asarray(inputs["b2"])
    mask = np.asarray(inputs["padding_mask"])
    assert np.all(ln_g == 1.0) and np.all(ln_b == 0.0)
    assert np.all(mb == 0.0) and np.all(mg > 0.0)
    assert np.all(b1 == 0.0) and np.all(b2 == 0.0)
    assert np.all(mask == 1.0)
    W1 = np.asarray(inputs["W1"], np.float64)
    W2 = np.asarray(inputs["W2"], np.float64)
    w1c = W1 - W1.sum(axis=2, keepdims=True) / H2           # [L, H, H2]
    w2g = np.asarray(mg, np.float64)[:, :, None] * W2        # [L, H2, H]
    gram = np.einsum("lif,ljf->lij", w1c, w1c)               # [L, H, H]
    w1c_l = np.ascontiguousarray(
        w1c.reshape(L, 2, 128, H2).transpose(2, 0, 1, 3).reshape(128, L * 2, H2)
    ).astype(BF16)
    w2g_l = np.ascontiguousarray(
        w2g.reshape(L, 4, 128, H).transpose(2, 0, 1, 3).reshape(128, L * 4, H)
    ).astype(BF16)
    gram_l = np.ascontiguousarray(
        gram.reshape(L, 2, 128, H).transpose(2, 0, 1, 3).reshape(128, L * 2, H)
    ).astype(BF16)
    t = np.asarray(inputs["t"], np.float32)
    return w1c_l, w2g_l, gram_l, t


# --------------------------------------------------------------- bass builder
def _build(ep, t_vals, layers=L):
    from contextlib import ExitStack
    import concourse.bacc as bacc
    import concourse.tile as tile
    from concourse import mybir
    from concourse.masks import make_identity

    f32 = mybir.dt.float32
    bf16 = mybir.dt.bfloat16
    i16 = mybir.dt.int16
    Alu = mybir.AluOpType
    Act = mybir.ActivationFunctionType

    gn = ep["gn"]
    gnh = ep["gnh"]
    sub_off = ep["sub_off"]
    TS = ep["total_sub"]
    TSLOT = TS * P
    nmats = ep["nmats"]
    g_mat_off = ep["g_mat_off"]
    chunk_mats = ep["chunk_mats"]
    N2 = N // 2

    nc = bacc.Bacc("TRN2", target_bir_lowering=False, debug=False, num_devices=8)

    x_in = nc.dram_tensor("x_in", [N, H], f32, kind="ExternalInput").ap()
    amat_d = nc.dram_tensor("amat", [P, nmats, P], bf16, kind="ExternalInput").ap()
    idxw_d = nc.dram_tensor("idxw", [P, TSLOT // 16], i16, kind="ExternalInput").ap()
    w1c_d = nc.dram_tensor("w1c", [P, L * 2, H2], bf16, kind="ExternalInput").ap()
    w2g_d = nc.dram_tensor("w2g", [P, L * 4, H], bf16, kind="ExternalInput").ap()
    gram_d = nc.dram_tensor("gram", [P, L * 2, H], bf16, kind="ExternalInput").ap()
    x_out = nc.dram_tensor("x_out", [N, H], f32, kind="ExternalOutput").ap()
    rbufs = [nc.dram_tensor(f"rbuf{i}", [N, H2], bf16).ap() for i in range(2)]

    x_in_t = x_in.rearrange("(c p) h -> p c h", p=P)    # [128, 64, 256]
    x_out_t = x_out.rearrange("(c p) h -> p c h", p=P)
    rbuf_ts = [rb.rearrange("(c p) h -> p c h", p=P) for rb in rbufs]

    with tile.TileContext(nc) as tc, ExitStack() as ctx:
        singles = ctx.enter_context(tc.tile_pool(name="singles", bufs=1))
        p_stat = ctx.enter_context(tc.tile_pool(name="stat", bufs=2))
        p_xl = ctx.enter_context(tc.tile_pool(name="xl", bufs=2))
        p_am = ctx.enter_context(tc.tile_pool(name="am", bufs=2))
        p_m = ctx.enter_context(tc.tile_pool(name="m", bufs=2))
        p_emm = ctx.enter_context(tc.tile_pool(name="emm", bufs=2))
        p_et = ctx.enter_context(tc.tile_pool(name="et", bufs=4))
        p_r1 = ctx.enter_context(tc.tile_pool(name="r1", bufs=2))
        p_agg = ctx.enter_context(tc.tile_pool(name="agg", bufs=2))
        p_z = ctx.enter_context(tc.tile_pool(name="z", bufs=2))
        p_zT = ctx.enter_context(tc.tile_pool(name="zT", bufs=2))
        p_ucT = ctx.enter_context(tc.tile_pool(name="ucT", bufs=2))
        p_rr = ctx.enter_context(tc.tile_pool(name="rr", bufs=2))
        p_or = ctx.enter_context(tc.tile_pool(name="or", bufs=2))
        p_xn = ctx.enter_context(tc.tile_pool(name="xn", bufs=1))
        ps_S = ctx.enter_context(tc.tile_pool(name="psS", bufs=2, space="PSUM"))
        ps_zT = ctx.enter_context(tc.tile_pool(name="pszT", bufs=1, space="PSUM"))
        ps_yc = ctx.enter_context(tc.tile_pool(name="psyc", bufs=1, space="PSUM"))
        ps_zM = ctx.enter_context(tc.tile_pool(name="pszM", bufs=1, space="PSUM"))
        ps_o = ctx.enter_context(tc.tile_pool(name="pso", bufs=1, space="PSUM"))

        x_sb = singles.tile([P, NCHUNK, H], bf16)
        h_sb = singles.tile([P, NCHUNK, H], bf16)
        idx_sb = singles.tile([P, TSLOT // 16], i16)
        w1c_sb = singles.tile([P, L * 2, H2], bf16)
        w2g_sb = singles.tile([P, L * 4, H], bf16)
        gram_sb = singles.tile([P, L * 2, H], bf16)
        ident = singles.tile([P, P], bf16)
        epsmat = singles.tile([P, P], bf16)
        maskrow = singles.tile([P, H2], bf16)
        eps_t = singles.tile([P, 1], f32)
        zero_t = singles.tile([P, 1], f32)

        for r in range(NGRP):
            xin_f = p_xl.tile([P, GRP, H], f32, tag="xstage")
            nc.sync.dma_start(out=xin_f[:], in_=x_in_t[:, r * GRP:(r + 1) * GRP, :])
            nc.scalar.copy(out=x_sb[:, r * GRP:(r + 1) * GRP, :], in_=xin_f[:])
        nc.sync.dma_start(out=idx_sb[:], in_=idxw_d)
        nc.sync.dma_start(out=w1c_sb[:], in_=w1c_d)
        nc.sync.dma_start(out=w2g_sb[:], in_=w2g_d)
        nc.sync.dma_start(out=gram_sb[:], in_=gram_d)
        make_identity(nc, ident[:])
        nc.vector.memset(epsmat[:], EPS_V)
        nc.vector.memset(maskrow[:, 0:H], 1.0)
        nc.vector.memset(maskrow[:, H:H2], 0.0)
        nc.vector.memset(eps_t[:], LN_EPS)
        nc.vector.memset(zero_t[:], 0.0)

        def node_sb(l, r):
            """LN stats + h + messages for chunks [4r, 4r+4), layer l."""
            t_l = float(t_vals[l])
            c0 = r * GRP
            st6 = p_stat.tile([P, GRP, 6], f32, tag="st6")
            for j in range(GRP):
                nc.vector.bn_stats(out=st6[:, j, :], in_=x_sb[:, c0 + j, :])
            mv = p_stat.tile([P, GRP, 2], f32, tag="mv")
            for j in range(GRP):
                nc.vector.bn_aggr(out=mv[:, j, :], in_=st6[:, j:j + 1, :])
            sd = p_stat.tile([P, GRP], f32, tag="sd")
            nc.scalar.activation(out=sd[:], in_=mv[:, :, 1], func=Act.Sqrt,
                                 bias=eps_t[:], scale=1.0)
            rln = p_stat.tile([P, GRP], f32, tag="rln")
            nc.vector.reciprocal(out=rln[:], in_=sd[:])
            mur = p_stat.tile([P, GRP], f32, tag="mur")
            nc.vector.tensor_tensor(out=mur[:], in0=mv[:, :, 0], in1=rln[:],
                                    op=Alu.mult)
            negmur = p_stat.tile([P, GRP], f32, tag="nmu")
            nc.vector.tensor_scalar(out=negmur[:], in0=mur[:],
                                    scalar1=-1.0, scalar2=0.0,
                                    op0=Alu.mult, op1=Alu.add)
            for j in range(GRP):
                c = c0 + j
                nc.scalar.activation(out=h_sb[:, c, :], in_=x_sb[:, c, :],
                                     func=Act.Identity,
                                     bias=negmur[:, j:j + 1],
                                     scale=rln[:, j:j + 1])
            m_t = p_m.tile([P, GRP, H], bf16, tag="m")
            nc.scalar.activation(out=m_t[:], in_=h_sb[:, c0:c0 + GRP, :],
                                 func=Act.Relu, bias=zero_t[:], scale=1.0)
            em_t = p_emm.tile([P, GRP, H], bf16, tag="em")
            nc.scalar.activation(out=em_t[:], in_=m_t[:], func=Act.Exp,
                                 bias=zero_t[:], scale=t_l)
            mm_t = p_emm.tile([P, GRP, H], bf16, tag="mm")
            nc.vector.tensor_tensor(out=mm_t[:], in0=m_t[:],
                                    in1=em_t[:], op=Alu.mult)
            nc.sync.dma_start(out=rbuf_ts[l % 2][:, c0:c0 + GRP, 0:H],
                              in_=em_t[:])
            nc.sync.dma_start(out=rbuf_ts[l % 2][:, c0:c0 + GRP, H:H2],
                              in_=mm_t[:])

        def edge_mlp(l, r):
            """Gather + scatter-matmul softmax agg + MLP for group r."""
            c0 = r * GRP
            g_n = gn[r]
            so = sub_off[r]
            nm = g_mat_off[r + 1] - g_mat_off[r]
            amg = p_am.tile([P, nm, P], bf16, tag="amg")
            nc.sync.dma_start(out=amg[:],
                              in_=amat_d[:, g_mat_off[r]:g_mat_off[r + 1], :])
            et = p_et.tile([P, g_n, H2], bf16, tag="et")
            gA, gB = gnh[r]
            if gB:
                nc.gpsimd.dma_gather(
                    et[:, 0:gA, :], rbufs[l % 2][0:N2, :],
                    idx_sb[:, so * 8:(so + gA) * 8],
                    num_idxs=gA * P, num_idxs_reg=gA * P, elem_size=H2,
                    single_packet=False)
                nc.gpsimd.dma_gather(
                    et[:, gA:g_n, :], rbufs[l % 2][N2:N, :],
                    idx_sb[:, (so + gA) * 8:(so + g_n) * 8],
                    num_idxs=gB * P, num_idxs_reg=gB * P, elem_size=H2,
                    single_packet=False)
            else:
                nc.gpsimd.dma_gather(
                    et[:], rbufs[l % 2], idx_sb[:, so * 8:(so + g_n) * 8],
                    num_idxs=g_n * P, num_idxs_reg=g_n * P, elem_size=H2,
                    single_packet=False)

            ag = p_agg.tile([P, GRP, H], bf16, tag="ag")
            for qp in range(GRP // 2):
                Sp = ps_S.tile([P, 2, H2], f32, tag="S")
                for q2 in range(2):
                    c = c0 + qp * 2 + q2
                    mats = chunk_mats[c]
                    nc.tensor.matmul(out=Sp[:, q2, :], lhsT=epsmat[:],
                                     rhs=maskrow[:], start=True,
                                     stop=(len(mats) == 0))
                    for k, (mi, si) in enumerate(mats):
                        nc.tensor.matmul(out=Sp[:, q2, :], lhsT=amg[:, mi, :],
                                         rhs=et[:, si, :],
                                         start=False, stop=(k == len(mats) - 1))
                r1 = p_r1.tile([P, 2, H], f32, tag="r1")
                nc.vector.reciprocal_approx_fast(out=r1[:], in_=Sp[:, :, 0:H])
                nc.vector.tensor_tensor(out=ag[:, qp * 2:qp * 2 + 2, :],
                                        in0=Sp[:, :, H:H2],
                                        in1=r1[:], op=Alu.mult)
            z_t = p_z.tile([P, GRP, H], bf16, tag="z")
            nc.vector.tensor_tensor(out=z_t[:], in0=ag[:],
                                    in1=h_sb[:, c0:c0 + GRP, :], op=Alu.add)

            # transpose z -> zT [128f, 2ki, 512n]
            zT = p_zT.tile([P, 2, GSZ], bf16, tag="zT")
            zp = ps_zT.tile([P, 2, GSZ], bf16, tag="zp")
            for q in range(GRP):
                for fb in range(2):
                    nc.tensor.transpose(out=zp[:, fb, q * P:(q + 1) * P],
                                        in_=z_t[:, q, fb * P:(fb + 1) * P],
                                        identity=ident[:])
            nc.vector.tensor_copy(out=zT[:], in_=zp[:])

            # mm1 + relu
            ucT = p_ucT.tile([P, 4, GSZ], bf16, tag="ucT")
            for fo in range(4):
                ycp = ps_yc.tile([P, GSZ], f32, tag="yc")
                for ki in range(2):
                    nc.tensor.matmul(
                        out=ycp[:], lhsT=w1c_sb[:, l * 2 + ki,
                                                fo * P:(fo + 1) * P],
                        rhs=zT[:, ki, :], start=(ki == 0), stop=(ki == 1))
                nc.scalar.activation(out=ucT[:, fo, :], in_=ycp[:],
                                     func=Act.Relu, bias=zero_t[:], scale=1.0)

            # var via Gram: var = rowdot(z @ M, z)/H2 ; rinv = 1/sqrt(var+eps)
            varr = p_rr.tile([P, GRP], f32, tag="varr")
            for q in range(GRP):
                zMp = ps_zM.tile([P, H], f32, tag="zM")
                for ki in range(2):
                    nc.tensor.matmul(out=zMp[:],
                                     lhsT=zT[:, ki, q * P:(q + 1) * P],
                                     rhs=gram_sb[:, l * 2 + ki, :],
                                     start=(ki == 0), stop=(ki == 1))
                scr = p_r1.tile([P, H], bf16, tag="scrd")
                nc.vector.tensor_tensor(out=scr[:], in0=zMp[:],
                                        in1=z_t[:, q, :], op=Alu.mult)
                scr2 = p_r1.tile([P, H], bf16, tag="scrd2")
                nc.vector.tensor_scalar(out=scr2[:], in0=scr[:],
                                        scalar1=1.0, scalar2=0.0,
                                        op0=Alu.mult, op1=Alu.add,
                                        accum_out=varr[:, q:q + 1])
            vabs = p_rr.tile([P, GRP], f32, tag="vabs")
            nc.vector.tensor_scalar_max(out=vabs[:], in0=varr[:], scalar1=0.0)
            sdv = p_rr.tile([P, GRP], f32, tag="sdv")
            nc.scalar.activation(out=sdv[:], in_=vabs[:], func=Act.Sqrt,
                                 bias=eps_t[:], scale=1.0 / H2)
            rinv = p_rr.tile([P, GRP], f32, tag="rinv")
            nc.vector.reciprocal(out=rinv[:], in_=sdv[:])

            # mm2 + o*r
            or_t = p_or.tile([P, GRP, H], bf16, tag="or")
            for q in range(GRP):
                op = ps_o.tile([P, H], f32, tag="o")
                for ki in range(4):
                    nc.tensor.matmul(out=op[:],
                                     lhsT=ucT[:, ki, q * P:(q + 1) * P],
                                     rhs=w2g_sb[:, l * 4 + ki, :],
                                     start=(ki == 0), stop=(ki == 3))
                nc.scalar.activation(out=or_t[:, q, :], in_=op[:],
                                     func=Act.Copy,
                                     scale=rinv[:, q:q + 1])
            # residual
            if l < layers - 1:
                xn = p_xn.tile([P, GRP, H], bf16, tag="xn")
                nc.vector.tensor_tensor(out=xn[:], in0=x_sb[:, c0:c0 + GRP, :],
                                        in1=or_t[:], op=Alu.add)
                nc.vector.tensor_scalar_max(out=x_sb[:, c0:c0 + GRP, :],
                                            in0=xn[:], scalar1=0.0)
            else:
                xf = p_xn.tile([P, GRP, H], f32, tag="xf")
                nc.vector.tensor_tensor(out=xf[:], in0=x_sb[:, c0:c0 + GRP, :],
                                        in1=or_t[:], op=Alu.add)
                nc.sync.dma_start(out=x_out_t[:, c0:c0 + GRP, :], in_=xf[:])

        for r in range(NGRP):
            node_sb(0, r)
        for l in range(layers):
            for r in range(NGRP):
                edge_mlp(l, r)
                if l + 1 < layers:
                    node_sb(l + 1, r)

    nc.compile()
    return nc


# ------------------------------------------------------------------ fallback
def _reference_numpy(inputs):
    x = np.asarray(inputs["input_nodes"], np.float64)
    edges = np.asarray(inputs["input_edges"], np.int64)
    mask = np.asarray(inputs["padding_mask"], np.float64)
    out = np.zeros_like(x)
    for g in range(G):
        xg = x[g].copy()
        src, dst = edges[g, 0], edges[g, 1]
        for l in range(L):
            ln_g = np.asarray(inputs["ln_g"][l], np.float64)
            ln_b = np.asarray(inputs["ln_b"][l], np.float64)
            t = float(inputs["t"][l])
            W1 = np.asarray(inputs["W1"][l], np.float64)
            b1 = np.asarray(inputs["b1"][l], np.float64)
            mg = np.asarray(inputs["mlp_ln_g"][l], np.float64)
            mb = np.asarray(inputs["mlp_ln_b"][l], np.float64)
            W2 = np.asarray(inputs["W2"][l], np.float64)
            b2 = np.asarray(inputs["b2"][l], np.float64)

            def ln(v, g_, b_):
                mu = v.mean(-1, keepdims=True)
                va = v.var(-1, keepdims=True)
                return (v - mu) / np.sqrt(va + 1e-5) * g_ + b_

            h = ln(xg, ln_g, ln_b)
            m = np.maximum(h[src], 0.0) + EPS
            logits = m * t
            mx = np.full((N, H), -np.inf)
            np.maximum.at(mx, dst, logits)
            mx = np.where(np.isfinite(mx), mx, 0.0)
            ex = np.exp(logits - mx[dst])
            den = np.zeros((N, H))
            np.add.at(den, dst, ex)
            alpha = ex / np.maximum(den[dst], EPS)
            agg = np.zeros((N, H))
            np.add.at(agg, dst, m * alpha)
            z = agg + h
            z = np.maximum(ln(z @ W1 + b1, mg, mb), 0.0) @ W2 + b2
            xg = (xg + z) * mask[g][:, None]
            if l < L - 1:
                xg = np.maximum(xg, 0.0)
        out[g] = xg
    return out.astype(np.float32)


# -------------------------------------------------------------------- kernel

_NEFF_CACHE_INSTALLED = False


def _install_neff_disk_cache(cache_dir="/root/.bass_neff_cache"):
    """Memoize the bass_exec custom-call neuronx_cc compile on disk so a
    fresh process reuses the compiled NEFF instead of re-running walrus."""
    global _NEFF_CACHE_INSTALLED
    if _NEFF_CACHE_INSTALLED:
        return
    import hashlib, os, pickle
    from concourse import bass2jax
    bass2jax.install_neuronx_cc_hook()
    try:
        import libneuronxla
    except ImportError:
        return
    os.makedirs(cache_dir, exist_ok=True)
    inner = libneuronxla.neuronx_cc

    def cached(code, code_format, platform_version, file_prefix):
        if b"bass_exec" not in code:
            return inner(code, code_format, platform_version, file_prefix)
        key = hashlib.sha256(code).hexdigest()
        path = os.path.join(cache_dir, key + ".pkl")
        if os.path.exists(path):
            try:
                with open(path, "rb") as f:
                    return pickle.load(f)
            except Exception:
                pass
        r = inner(code, code_format, platform_version, file_prefix)
        try:
            tmp = path + ".tmp"
            with open(tmp, "wb") as f:
                pickle.dump(r, f)
            os.replace(tmp, path)
        except Exception:
            pass
        return r

    libneuronxla.neuronx_cc = cached
    _NEFF_CACHE_INSTALLED = True


_CACHE = {}


def kernel(**inputs):
    try:
        w1c_l, w2g_l, gram_l, t = _prep_weights(inputs)
    except AssertionError:
        return _reference_numpy(inputs)

    ep = _prep_edges(np.asarray(inputs["input_edges"]))
    key = (tuple(ep["gn"]),
           tuple(tuple(map(tuple, cm)) for cm in ep["chunk_mats"]),
           tuple(map(float, t)))
    if key not in _CACHE:
        _CACHE.clear()
        _CACHE[key] = _build(ep, t)
    nc = _CACHE[key]

    from concourse.bass_utils import run_bass_kernel_spmd
    _install_neff_disk_cache()
    x = np.ascontiguousarray(np.asarray(inputs["input_nodes"], np.float32))
    amat_l = np.ascontiguousarray(ep["amat"])
    idxw_l = np.ascontiguousarray(ep["idxw"])
    in_maps = []
    for g in range(G):
        in_maps.append({
            "x_in": x[g],
            "amat": amat_l[g],
            "idxw": idxw_l[g],
            "w1c": w1c_l,
            "w2g": w2g_l,
            "gram": gram_l,
        })
    res = run_bass_kernel_spmd(nc, in_maps, core_ids=list(range(G)))
    out = np.stack([np.asarray(res.results[g]["x_out"]) for g in range(G)])
    return out.astype(np.float32)


if __name__ == "__main__":
    sys.path.insert(0, "/root/problem")
    import reference
    inputs = {k: np.asarray(v) for k, v in reference.setup_inputs().items()}
    got = kernel(**inputs)
    expected = np.asarray(reference.reference(**inputs))
    rel = np.linalg.norm(got - expected) / np.linalg.norm(expected)
    print("rel l2 err:", rel)
